# revision 18
# baseline (speedup 1.0000x reference)
"""Trainium2 Bass kernel for nn_Codec_41798621725069.

The reference runs a T=16 encode/decode scan, but the float arithmetic
collapses exactly:

  encode: f0=0, lr0=1  ->  spike_0 = 0.5*(1-x), f1 = x (exact);
          every later gradient is exactly 0, so spike_t = 0.5 for t>=1.
  decode: y0=0, lr0=1  ->  y1 = -(2*spike_0 - 1) = -((1-x) - 1);
          every later decode gradient is exactly 0.

So y = -(fl(fl(1-x) - 1)) elementwise in f32, which by sign-symmetry of
round-to-nearest equals fl(fl(x-1) + 1) -- one DVE tensor_scalar
(subtract 1, add 1) per element, bit-exact with the reference.

Sharding: pure data parallel -- each of the 8 cores streams a contiguous
1/8 slice of x (1M elements = 4 MiB) through SBUF and back.

Raw Bass (no TileContext): this toolchain's walrus lowering allows very
few embedded sem-waits per instruction (1 on a DMA), which Tile's
auto-generated sync (and its kernel-tail drain) exceeds.  With explicit
semaphores every wait is a standalone sequencer instruction: loads
stream on the SP HWDGE ring with a per-tile completion semaphore,
stores on the Activation HWDGE ring so the two directions overlap.
"""

import numpy as np

N = 8388608
NCORES = 8
SHARD = N // NCORES          # 1048576 elements per core
P = 128                      # SBUF partitions
COLS = SHARD // P            # 8192 f32 per partition (32 KiB)
# Pipeline chunk widths (columns).  Few big tiles amortize the ~0.7us
# per-DMA issue cost; the tiny tiles at the end keep the serial tail
# (last load -> DVE -> last store -> completion receipt) short.
TILE_SPLIT = [1984, 1984, 1984, 1984, 192, 64]

_cache = {}
last_results = None          # BassKernelResults from the most recent run


def _build_nc(split=None, load_rings=("sync",), store_rings=("scalar",),
              barrier="evsem", dma_reset=True):
    from contextlib import ExitStack

    import concourse.bass as bass
    import concourse.mybir as mybir

    f32 = mybir.dt.float32
    # Bass.__init__ unconditionally emits a const-pool init (4 memsets
    # nothing here reads) plus an all-engine barrier (~0.5us of kernel
    # entry).  Suppress both during construction only -- the sem-clear
    # barrier below provides the one cross-engine sync this kernel needs.
    orig_init = bass.Bass.__init__
    orig_barrier = bass.Bass.all_engine_barrier
    orig_memset = bass.BassSharedVectorInterface.memset

    def patched_init(self, *a, **k):
        bass.Bass.all_engine_barrier = lambda s, **kk: None
        bass.BassSharedVectorInterface.memset = lambda s, ap, c: None
        try:
            orig_init(self, *a, **k)
        finally:
            bass.Bass.all_engine_barrier = orig_barrier
            bass.BassSharedVectorInterface.memset = orig_memset

    bass.Bass.__init__ = patched_init
    try:
        nc = bass.Bass()
    finally:
        bass.Bass.__init__ = orig_init
    x = nc.declare_dram_parameter("x", [P, COLS], f32, isOutput=False)
    out = nc.declare_dram_parameter("out", [P, COLS], f32, isOutput=True)

    split = list(split if split is not None else TILE_SPLIT)
    assert sum(split) == COLS
    n = len(split)
    offs = [sum(split[:i]) for i in range(n)]
    engines = {"sync": nc.sync, "scalar": nc.scalar, "gpsimd": nc.gpsimd}

    with ExitStack() as ctx:
        t_in = ctx.enter_context(nc.sbuf_tensor("t_in", [P, COLS], f32))
        t_out = ctx.enter_context(nc.sbuf_tensor("t_out", [P, COLS], f32))
        # One completion sem per load tile: a DMA's 16 SDMA engines each
        # inc by 1 as they finish their 8-partition slice, so with a single
        # shared sem an intermediate threshold 16*(i+1) can be reached by
        # engine-skewed partial sums while tile i is still in flight.  Only
        # a per-DMA sem (wait ==16) or the full-stream total is sound.
        load_sems = [
            ctx.enter_context(nc.semaphore(f"load_sem{i}")) for i in range(n)
        ]
        dve_sem = ctx.enter_context(nc.semaphore("dve_sem"))
        store_sem = ctx.enter_context(nc.semaphore("store_sem"))
        # No nc.Block(): its exit path appends per-engine drains plus an
        # all-engine barrier (~1us of tail).  Engine streams may simply end;
        # the final store_sem wait below keeps the program alive until the
        # last byte lands, and the next execution's entry sync realigns the
        # engines.

        # A re-execution of this NEFF starts with these sem indices at
        # their previous end values, which would let waits below fall
        # through immediately.  Reset them, then barrier so no engine
        # touches a sem before the clear lands.
        sems = sorted(s.num for s in (*load_sems, dve_sem, store_sem))
        assert sems[-1] - sems[0] == len(sems) - 1, sems
        if dma_reset:
            nc.gpsimd.dma_reset(range(sems[0], sems[-1] + 1))
        nc.gpsimd.sem_clear(range(sems[0], sems[-1] + 1))
        if barrier == "evsem":
            nc.all_engine_barrier()
        else:
            nc._nrt_pseudo_barrier()

        for i in range(n):
            cs = slice(offs[i], offs[i] + split[i])
            eng = engines[load_rings[i % len(load_rings)]]
            eng.dma_start(out=t_in[:, cs], in_=x[:, cs]).then_inc(load_sems[i], 16)

        for i in range(n):
            cs = slice(offs[i], offs[i] + split[i])
            nc.vector.wait_ge(load_sems[i], 16)
            # y = (x - 1) + 1 with both roundings, matching the
            # reference's -( (1-x) - 1 ) bit-for-bit.
            nc.vector.tensor_scalar(
                out=t_out[:, cs],
                in0=t_in[:, cs],
                scalar1=1.0,
                scalar2=1.0,
                op0=mybir.AluOpType.subtract,
                op1=mybir.AluOpType.add,
            ).then_inc(dve_sem, 1)

        for i in range(n):
            cs = slice(offs[i], offs[i] + split[i])
            eng = engines[store_rings[i % len(store_rings)]]
            eng.wait_ge(dve_sem, i + 1)
            eng.dma_start(out=out[:, cs], in_=t_out[:, cs]).then_inc(store_sem, 16)
        # Full-stream total: sound on a shared sem, and guarantees the last
        # byte has landed in HBM before the program ends.  Every store ring
        # waits so no engine's stream retires before its stores landed.
        for ring in dict.fromkeys(store_rings):
            engines[ring].wait_ge(store_sem, 16 * n)

    return nc


def _get_nc():
    if "nc" not in _cache:
        _cache["nc"] = _build_nc()
    return _cache["nc"]


def kernel(x: np.ndarray) -> np.ndarray:
    global last_results
    from concourse.bass_utils import run_bass_kernel_spmd

    x = np.ascontiguousarray(x, dtype=np.float32)
    assert x.shape == (N,), x.shape

    shards = x.reshape(NCORES, P, COLS)
    in_maps = [{"x": shards[i]} for i in range(NCORES)]

    nc = _get_nc()
    last_results = run_bass_kernel_spmd(nc, in_maps, core_ids=list(range(NCORES)))

    outs = [last_results.results[i]["out"].reshape(-1) for i in range(NCORES)]
    return np.concatenate(outs).astype(np.float32, copy=False)
